# revision 3
# baseline (speedup 1.0000x reference)
"""Self-contained TRN2 Bass kernel for nn_FLoRALayer (B=8, S=2048, D=1024, R=8).

kernel(**inputs) takes FULL unsharded inputs:
    x         [8, 2048, 1024] f32
    adapter_b [8, 1024, 8]    f32
    adapter_a [8, 8, 1024]    f32
    W0        [1024, 1024]    f32
returns the FULL [8, 2048, 1024] f32 output of:
    BxW0 = einsum('bsd,bdr,do->bsro', x.astype(fp16), adapter_b, W0)
    out  = relu(mean(swapaxes(adapter_a,1,2)[:,None]*BxW0.reshape(b,s,d,r), -1))

Math refactor (verified exactly): with o = kk*128 + g*16 + mp,
    W_eff[dd, o] = adapter_b[dd, kk] * sum_rp adapter_a[rp, o] * W0[dd, (o%128)*8 + rp]
    out[b] = relu((x_fp16[b] @ W_eff[b]) / 8)
which is one [2048,1024] @ [1024,1024] matmul per batch -- data-parallel over
the batch dim: batch b runs on NeuronCore b (sharding_hint's layout).

Host does ONLY data placement (sharding/permutation/replication, no
arithmetic): X^T and W0^T tile-packing, a block-diagonal embedding of
adapter_a (A_sp), and 16x replication of adapter_b (B_bc).

Device per core:
  - W0^T tiles: DMA f32 -> DVE cast fp16
  - C[dd-chunk, (kk,mp)-block g] = W0T_g_chunk.T @ A_sp[g] on the PE
    (A_sp is block-diagonal so the single K=128 matmul per (t,g) performs the
    grouped rank-8 contraction), then W_eff = C * B_bc on DVE (stride-0
    broadcast tensor_tensor), scattered into true-o column order, fp16
  - x tiles: DMA f32 -> DVE cast fp16 (X^T layout comes from host packing)
  - main: PSUM[s,o] += X^T_chunk.T @ W_eff_chunk (fp16, N=512) over 8 chunks;
    3 "warm" s-tiles run interleaved with the W0 chain to hide the W0 DMA
  - evac relu(0.125*psum): one half on ACT, one on DVE; stores alternate
    sync/SWDGE queues
"""

from contextlib import ExitStack

import numpy as np

F32 = None  # set after imports inside kernel build (mybir)

S, D, R = 2048, 1024, 8
NT = D // 128
NS = S // 128
WARM = [0, 1, 2]
N_CORES = 8

_compiled = None


def _build_kernel():
    import concourse.bass as bass
    import concourse.tile as tile
    from concourse import bacc, mybir

    F32 = mybir.dt.float32
    F16 = mybir.dt.float16

    nc = bacc.Bacc(
        "TRN2", target_bir_lowering=False, debug=False, num_devices=N_CORES
    )

    x_d = nc.dram_tensor("xtp", [NS, 128, D], F32, kind="ExternalInput").ap()
    w0_d = nc.dram_tensor("w0tp", [128, NT * D], F32, kind="ExternalInput").ap()
    asp_d = nc.dram_tensor("asp", [128, NT * 128], F32, kind="ExternalInput").ap()
    bbc_d = nc.dram_tensor("bbc", [128, NT * 128], F32, kind="ExternalInput").ap()
    out_d = nc.dram_tensor("out", [S, D], F32, kind="ExternalOutput").ap()

    with tile.TileContext(nc) as tc, ExitStack() as ctx:
        pool = lambda name, bufs, **kw: ctx.enter_context(
            tc.tile_pool(name=name, bufs=bufs, **kw)
        )
        const_p = pool("const", 1)
        w0stage_p = pool("w0stage", 1)
        w0t_p = pool("w0t", 1)
        weff_p = pool("weff", 1)
        xstage_p = pool("xstage", 4)
        xth_p = pool("xth", 5)
        outst_p = pool("outst", 4)
        pmm_p = pool("pmm", 8, space="PSUM")

        xth_tiles = {}

        def x_load_cast(s):
            xs = xstage_p.tile([128, D], F32, tag="xs", name=f"xs{s}")
            nc.sync.dma_start(xs[:], x_d[s])
            xth = xth_p.tile([128, D], F16, tag="xth", name=f"xth{s}")
            nc.vector.tensor_copy(xth[:], xs[:])
            xth_tiles[s] = xth

        x_load_cast(0)

        asp_st = const_p.tile([128, NT * 128], F32, tag="asp_st")
        nc.sync.dma_start(asp_st[:], asp_d[:])
        asp_h = const_p.tile([128, NT * 128], F16, tag="asp_h")
        nc.vector.tensor_copy(asp_h[:], asp_st[:])
        bbc = const_p.tile([128, NT * 128], F32, tag="bbc")
        nc.sync.dma_start(bbc[:], bbc_d[:])

        w0t = w0t_p.tile([128, NT * 1024], F16, tag="w0t")
        weff = weff_p.tile([128, NT * 1024], F16, tag="weff")

        # partition-major packed W0: 1MB contiguous-per-partition DMA chunks
        w0s = w0stage_p.tile([128, NT * D], F32, tag="w0s")

        def w0_dma(q):
            nc.scalar.dma_start(
                w0s[:, q * 2 * D : (q + 1) * 2 * D],
                w0_d[:, q * 2 * D : (q + 1) * 2 * D],
            )

        for s in WARM[1:]:
            x_load_cast(s)

        po_warm = {
            s: [
                pmm_p.tile([128, 512], F32, tag="pmm", name=f"po{s}_{i}")
                for i in range(2)
            ]
            for s in WARM
        }

        def warm_mms(c):
            for s in WARM:
                for h in range(2):
                    nc.tensor.matmul(
                        po_warm[s][h][:],
                        lhsT=xth_tiles[s][:, c * 128 : (c + 1) * 128],
                        rhs=weff[:, c * 1024 + h * 512 : c * 1024 + (h + 1) * 512],
                        start=(c == 0),
                        stop=(c == NT - 1),
                    )

        import concourse.mybir as mybir_mod

        w0_dma(0)
        w0_dma(1)
        for t in range(NT):
            if t % 2 == 0 and t // 2 + 2 < 4:
                w0_dma(t // 2 + 2)
            nc.vector.tensor_copy(
                w0t[:, t * 1024 : (t + 1) * 1024], w0s[:, t * D : (t + 1) * D]
            )
            pcs = [
                pmm_p.tile([128, 512], F32, tag="pmm", name=f"pc{t}_{i}")
                for i in range(2)
            ]
            for g in range(NT):
                nc.tensor.matmul(
                    pcs[g // 4][:, (g % 4) * 128 : (g % 4 + 1) * 128],
                    lhsT=w0t[:, t * 1024 + g * 128 : t * 1024 + (g + 1) * 128],
                    rhs=asp_h[:, g * 128 : (g + 1) * 128],
                    start=True,
                    stop=True,
                )
            wv = weff[:, t * 1024 : (t + 1) * 1024].rearrange(
                "p (kk g mp) -> p kk g mp", kk=8, g=NT, mp=16
            )
            for half in range(2):
                wvh = wv[:, :, half * 4 : (half + 1) * 4, :]
                bv = bbc[:, t * 128 : (t + 1) * 128].rearrange(
                    "p (kk mp) -> p kk mp", kk=8
                )[:, :, None, :].broadcast_to([128, 8, 4, 16])
                pv = pcs[half].rearrange("p (g kk mp) -> p kk g mp", g=4, kk=8)
                nc.vector.tensor_tensor(
                    out=wvh, in0=pv, in1=bv, op=mybir_mod.AluOpType.mult
                )
            # consume weff one chunk behind its construction: warm(t-1) only
            # needs BTT(t-1), which finished during C(t) -- no serial chain
            if t > 0:
                warm_mms(t - 1)
        warm_mms(NT - 1)

        x_load_cast(len(WARM))
        for s in range(NS):
            if s not in WARM:
                for sp in (s + 1, s + 2):
                    if sp < NS and sp not in xth_tiles:
                        x_load_cast(sp)
                xth = xth_tiles[s]
                po = [
                    pmm_p.tile([128, 512], F32, tag="pmm", name=f"po{s}_{i}")
                    for i in range(2)
                ]
                for c in range(NT):
                    for h in range(2):
                        nc.tensor.matmul(
                            po[h][:],
                            lhsT=xth[:, c * 128 : (c + 1) * 128],
                            rhs=weff[:, c * 1024 + h * 512 : c * 1024 + (h + 1) * 512],
                            start=(c == 0),
                            stop=(c == NT - 1),
                        )
            else:
                po = po_warm[s]
            outst = outst_p.tile([128, D], F32, tag="outst", name=f"outst{s}")
            nc.scalar.activation(
                outst[:, 0:512],
                po[0][:],
                mybir_mod.ActivationFunctionType.Relu,
                scale=0.125,
            )
            nc.vector.tensor_scalar(
                out=outst[:, 512:1024],
                in0=po[1][:],
                scalar1=0.125,
                scalar2=0.0,
                op0=mybir_mod.AluOpType.mult,
                op1=mybir_mod.AluOpType.max,
            )
            eng = nc.sync if s % 2 == 0 else nc.gpsimd
            eng.dma_start(out_d[s * 128 : (s + 1) * 128, :], outst[:])

    nc.compile()
    return nc


def _pack_inputs(x_b, adapter_b_b, adapter_a_b, W0):
    """Pure data placement (permutation / replication / zero-padding)."""
    xtp = np.ascontiguousarray(
        x_b.reshape(NS, 128, NT, 128).transpose(0, 3, 2, 1).reshape(NS, 128, D),
        np.float32,
    )
    w0tp = np.ascontiguousarray(
        W0.reshape(NT, 128, NT, 128).transpose(3, 0, 2, 1).reshape(128, NT * D),
        np.float32,
    )
    asp = np.zeros((NT, 128, 128), np.float32)
    aa = adapter_a_b
    for g in range(NT):
        for mp in range(16):
            for rp in range(R):
                asp[g, mp * 8 + rp, np.arange(8) * 16 + mp] = aa[
                    rp, np.arange(8) * 128 + g * 16 + mp
                ]
    asp = np.ascontiguousarray(asp.transpose(1, 0, 2).reshape(128, NT * 128))
    bbc = np.repeat(adapter_b_b, 16, axis=1).reshape(D, 128)
    bbc = np.ascontiguousarray(
        bbc.reshape(NT, 128, 128).transpose(1, 0, 2).reshape(128, NT * 128),
        np.float32,
    )
    return {"xtp": xtp, "w0tp": w0tp, "asp": asp, "bbc": bbc}


def kernel(x, adapter_b, adapter_a, W0):
    global _compiled
    x = np.asarray(x, np.float32)
    adapter_b = np.asarray(adapter_b, np.float32)
    adapter_a = np.asarray(adapter_a, np.float32)
    W0 = np.asarray(W0, np.float32)
    B = x.shape[0]
    assert B == N_CORES and x.shape == (B, S, D)

    if _compiled is None:
        _compiled = _build_kernel()

    from concourse.bass_utils import run_bass_kernel_spmd

    in_maps = [
        _pack_inputs(x[b], adapter_b[b], adapter_a[b], W0) for b in range(B)
    ]
    res = run_bass_kernel_spmd(_compiled, in_maps, list(range(N_CORES)))
    out = np.stack([res.results[b]["out"] for b in range(B)]).astype(np.float32)
    return out
